# revision 38
# baseline (speedup 1.0000x reference)
"""Trainium2 Bass kernel for batched attention (B=8, Lq=Lk=2048, D=Dv=128).

Sharding: pure data parallel - batch element b runs on NeuronCore b.

The device hot loop is ACT(exp)-bound at (172+1024)/1.2 ~= 1us per
[128k,1024q] tile, 32 tiles (~32us); everything else is scheduled to
hide under that stream. Measured ~52.3us end to end: ~7us runtime boot,
first exp at ~12.3us (gated by the first input DMA's completion
latency), 32.4us exp loop at 996ns/iter, ~3us tail, ~3.9us teardown.

Host prep (numpy, trivial vs the kernel): Q/K pre-transposed to [d, L]
bf16, V pre-tiled to [p, t, d] (k = t*128+p) bf16, W2^T = Wq @ Wk^T
(constant-folds both score projections into one weight), Wv bf16, mask
pre-converted to an additive exp bias, V's last k-tile additionally
pre-transposed (for the tail bypass). Inputs are concatenated into 5
"packs" DMA'd in need-by order on the sync HWDGE ring (FIFO per ring,
so the first-exp gate lands first and nothing competes for HBM early).

Device per-core:
  qT2 = W2T^T @ xqT           (2 matmuls on 2 psA tiles; evacuated in
                               parallel on ACT + DVE)
  per k-tile jl (x16, x2 q-halves):
    sT_j = xkT_j^T @ qT2      [128k, 1024q] PSUM, 3-slot psA rotation
    a_j  = exp(sT_j*s + bias) ACT, the bottleneck stream
    u   += xv_j^T @ a_j       [d, 1024q] PSUM accum; pops lag 2 iters
                               so their exp-dependency never blocks the
                               PE queue
    S   += a_j                DVE bf16 adds (softmax denominator)
  per-half epilogue: den = S^T @ 1 (8 tiny matmuls), recip on DVE,
  o = u^T @ Wv (8 matmuls), out = o * (1/den) broadcast, DMA out.

h0's epilogue is spread over loop slots j19..j24 (4 matmuls/iter so no
PE-queue burst delays a scores tile; den/o live on psA slots inserted
one per iteration to keep a >=2-slot rotation margin). h1's tail is
collapsed: its last k-tile (31) bypasses u - u1 closes at tile 30,
both u1 chunks evacuate on DVE during the last exp, o1_partial =
u1^T Wv and tile 31's correction a31^T @ (V31 Wv) form one complete
PSUM accumulation group per 128-chunk (zero-region rule: one open
group per 2KB bank), with V31 Wv precomputed mid-loop on the psU
chain. The denominator is likewise split: partial den over tiles
16..30 during j31 (bank A) + 8 finals on a31 (bank B) + a tiny DVE
add. After the final exp only ~3us remains before the last output DMA.

PSUM: banks 0-5 scores rotation (psA x3; also hosts qT2 staging,
dps0/o0 mid-loop and dps1/o1a/o1b in the tail); banks 6-7 serial psU
chain warmfill -> pps1 -> u0 -> vproj -> u1.

S15's DVE add is deferred behind the u0 evacuation + vproj copy so the
u1 region frees a slot earlier (S15 isn't needed until den0 at j19).
"""

import sys

sys.path.insert(0, "/opt/trn_rl_repo")

import numpy as np
import ml_dtypes

import concourse.bass as bass
import concourse.mybir as mybir
import concourse.tile as tile
from concourse import bacc
from concourse.bass_utils import run_bass_kernel_spmd

P = 128
L = 2048
D = 128
T = L // P  # 16 k-tiles
HQ = 1024  # q-half size
F32 = mybir.dt.float32
BF16 = mybir.dt.bfloat16
SCALE = 1.0 / float(np.sqrt(128.0))
N_CORES = 8

ADD = mybir.AluOpType.add
MULT = mybir.AluOpType.mult
EXP = mybir.ActivationFunctionType.Exp

BF16NP = ml_dtypes.bfloat16

# pack layouts (bf16 columns), in DMA (need-by) order
#   pack0:  w2T(128) | mb-as-bf16(32) | qT[:, 0:512]        = 672
#   pack1a: qT[:, 512:1024] | kT[:, 0:512] | v[:, 0:512]    = 1536
#   pack1b: qT[:, 1024:2048]                                = 1024
#   pack2:  kT[:, 512:1536] | v[:, 512:1024] | wv | vT31    = 1792
#   pack3:  kT[:, 1536:2048] | v[:, 1024:2048]              = 1536
PK0 = 128 + 32 + 512
PK1A = 512 + 512 + 512
PK1B = HQ
PK2 = 1024 + 512 + 128 + 128
PK3 = 512 + 1024


def build():
    nc = bacc.Bacc("TRN2", target_bir_lowering=False, debug=False)

    p0_ext = nc.declare_dram_parameter("pack0", [P, PK0], BF16, isOutput=False)
    p1a_ext = nc.declare_dram_parameter(
        "pack1a", [P, PK1A], BF16, isOutput=False
    )
    p1b_ext = nc.declare_dram_parameter(
        "pack1b", [P, PK1B], BF16, isOutput=False
    )
    p2_ext = nc.declare_dram_parameter("pack2", [P, PK2], BF16, isOutput=False)
    p3_ext = nc.declare_dram_parameter("pack3", [P, PK3], BF16, isOutput=False)
    out_ext = nc.declare_dram_parameter("out", [P, L], BF16, isOutput=True)

    with tile.TileContext(nc) as tc:
        with (
            tc.tile_pool(name="const", bufs=1) as const,
            tc.tile_pool(name="big", bufs=1) as big,
            tc.tile_pool(name="att", bufs=9) as att,
            # score rotation: 3 x [128,1024]f32 tiles (PSUM banks 0-5)
            tc.tile_pool(name="psA", bufs=3, space="PSUM") as psA,
            # serial chain (banks 6-7)
            tc.tile_pool(name="psU", bufs=1, space="PSUM") as psU,
        ):
            # ---- tiny init (DVE) + exp-table preload ----
            warm = const.tile([P, P], BF16, tag="warm")
            nc.vector.memset(warm[:], 0.125)
            ones_col = const.tile([P, 1], BF16, tag="ones")
            nc.vector.memset(ones_col[:], 1.0)
            dummy_exp = const.tile([P, 1], F32, tag="dummy")
            nc.scalar.activation(dummy_exp[:], warm[:, 0:1], EXP)

            # ---- packed input DMAs, all on the sync HWDGE ring ----
            pk0 = big.tile([P, PK0], BF16, tag="pk0")
            pk1a = big.tile([P, PK1A], BF16, tag="pk1a")
            pk1b = big.tile([P, PK1B], BF16, tag="pk1b")
            pk2 = big.tile([P, PK2], BF16, tag="pk2")
            pk3 = big.tile([P, PK3], BF16, tag="pk3")
            nc.sync.dma_start(pk0[:], p0_ext[:])
            nc.sync.dma_start(pk1a[:], p1a_ext[:])
            nc.sync.dma_start(pk1b[:], p1b_ext[:])
            nc.sync.dma_start(pk2[:], p2_ext[:])
            nc.sync.dma_start(pk3[:], p3_ext[:])

            w2T_bf = pk0[:, 0:128]
            mask_bias = pk0[:, 128:160].bitcast(F32)  # [P, 16] f32
            wv_bf = pk2[:, 1536:1664]
            vT31_sb = pk2[:, 1664:1792]

            def xq_cols(c0, c1):  # qT columns [c0:c1)
                if c1 <= 512:
                    return pk0[:, 160 + c0 : 160 + c1]
                if c1 <= HQ:
                    return pk1a[:, c0 - 512 : c1 - 512]
                return pk1b[:, c0 - HQ : c1 - HQ]

            def xk_tile(jl):  # kT columns [jl*128:(jl+1)*128)
                c = jl * P
                if c < 512:
                    return pk1a[:, 512 + c : 512 + c + P]
                if c < 1536:
                    return pk2[:, c - 512 : c - 512 + P]
                return pk3[:, c - 1536 : c - 1536 + P]

            def xv_tile(jl):  # v tile jl = rows jl*128..+127, [P(k), D]
                c = jl * P
                if c < 512:
                    return pk1a[:, 1024 + c : 1024 + c + P]
                if c < 1024:
                    return pk2[:, 1024 + c - 512 : 1024 + c - 512 + P]
                return pk3[:, 512 + c - 1024 : 512 + c - 1024 + P]

            # ---- PE warm-up fillers (HAM un-throttle before the chain) ----
            warmfill = psU.tile([P, 512], F32, tag="u", name="warmfill")

            def fillers(n):
                for _ in range(n):
                    nc.tensor.matmul(
                        warmfill[:, 0:P], warm[:], warm[:],
                        start=True, stop=True,
                    )

            fillers(32)

            # ---- qT2 = W2T^T @ xqT, half 0 (gates the loop start) ----
            # two independent psA tiles so the chunk-0 evac does not
            # serialize against the chunk-1 matmul (per-tile dep tracking)
            qT2 = big.tile([P, L], BF16, tag="qT2")
            with tc.high_priority():
                pps0a = psA.tile([P, HQ], F32, tag="sc", name="qp0a")
                pps0b = psA.tile([P, HQ], F32, tag="sc", name="qp0b")
                for c, pp in enumerate((pps0a, pps0b)):
                    nc.tensor.matmul(
                        pp[:, 0:512],
                        w2T_bf,
                        xq_cols(c * 512, (c + 1) * 512),
                        start=True,
                        stop=True,
                    )
                # evac chunk 0 on ACT, chunk 1 on DVE (parallel)
                nc.scalar.copy(out=qT2[:, 0:512], in_=pps0a[:, 0:512])
                nc.vector.tensor_copy(
                    out=qT2[:, 512:1024], in_=pps0b[:, 0:512]
                )

            # ---- main loop state ----
            S_h = [
                big.tile([P, HQ], BF16, tag=f"S{h}", name=f"S{h}")
                for h in range(2)
            ]
            u_bf0 = big.tile([P, HQ], BF16, tag="u_bf0", name="u_bf0")
            u1c = [
                big.tile([P, 512], BF16, tag=f"u1c{c}", name=f"u1c{c}")
                for c in range(2)
            ]

            def u_bf_chunk(h, c):  # [P,128] slice for o-matmul chunk c
                if h == 0:
                    return u_bf0[:, c * P : (c + 1) * P]
                return u1c[c // 4][:, (c % 4) * P : (c % 4 + 1) * P]
            vproj = big.tile([P, D], BF16, tag="vproj")
            out_all = big.tile([P, T, D], BF16, tag="out_all")
            out_dst = out_ext[:].rearrange("p (t d) -> p t d", t=T)

            def emit_scores(h, jl, sc):
                for c in range(2):
                    nc.tensor.matmul(
                        sc[:, c * 512 : (c + 1) * 512],
                        xk_tile(jl),
                        qT2[:, h * HQ + c * 512 : h * HQ + (c + 1) * 512],
                        start=True,
                        stop=True,
                    )

            def emit_u(u_ps, h, jl, a_t):
                last = T - 1 if h == 0 else T - 2
                for c in range(2):
                    nc.tensor.matmul(
                        u_ps[:, c * 512 : (c + 1) * 512],
                        xv_tile(jl),
                        a_t[:, c * 512 : (c + 1) * 512],
                        start=(jl == 0),
                        stop=(jl == last),
                    )

            def emit_S(h, jl, a_t):
                if jl == 0:
                    nc.vector.tensor_copy(out=S_h[h][:], in_=a_t[:])
                else:
                    nc.vector.tensor_tensor(S_h[h][:], S_h[h][:], a_t[:], ADD)

            u_ps = {}
            pend = []  # [(h, jl, a_tile)] u-matmul work lagged behind exp

            def pop_u(n):
                for _ in range(n):
                    if not pend:
                        return
                    ph, pj, pa = pend.pop(0)
                    if ph not in u_ps:
                        u_ps[ph] = psU.tile(
                            [P, HQ], F32, tag="u", name=f"u{ph}"
                        )
                    emit_u(u_ps[ph], ph, pj, pa)

            denT = [None, None]
            rT = [None, None]

            def den_mms(h, dps, start, stop, src, cols):
                # dps[:, c] (+)= sum over partitions of src[:, c*P:(c+1)*P]
                for c in range(cols):
                    nc.tensor.matmul(
                        dps[:, c : c + 1],
                        src[:, c * P : (c + 1) * P],
                        ones_col[:],
                        start=start,
                        stop=stop,
                    )

            def den_recip(h, dps):
                denT[h] = const.tile(
                    [P, 8], F32, tag=f"denT{h}", name=f"denT{h}"
                )
                nc.vector.tensor_copy(out=denT[h][:], in_=dps[:, 0:8])
                rT[h] = const.tile([P, 8], F32, tag=f"rT{h}", name=f"rT{h}")
                nc.vector.reciprocal(rT[h][:], denT[h][:])

            def o_mms(h, o_ps, c0, c1, start=True, stop=True):
                for c in range(c0, c1):
                    nc.tensor.matmul(
                        o_ps[:, c * P : (c + 1) * P],
                        u_bf_chunk(h, c),
                        wv_bf,
                        start=start,
                        stop=stop,
                    )

            def scale_out(h, o_ps, g, base=None):
                # out = o * r, r broadcast along dv; 4 q-chunks per call
                b = 4 * g * P if base is None else base
                nc.vector.tensor_tensor(
                    out_all[:, h * 8 + 4 * g : h * 8 + 4 * (g + 1), :],
                    o_ps[:, b : b + 4 * P].rearrange("p (c v) -> p c v", c=4),
                    rT[h][:, 4 * g : 4 * (g + 1)].to_broadcast([P, 4, P]),
                    MULT,
                )

            def out_dma(h, g, eng):
                eng.dma_start(
                    out_dst[:, h * 8 + 4 * g : h * 8 + 4 * (g + 1), :],
                    out_all[:, h * 8 + 4 * g : h * 8 + 4 * (g + 1), :],
                )

            # pop schedule: h0 tiles 0..14 at j=3..14 (2/iter at 3,4,5),
            # tile 15 flushed at j15; h1 tiles 16..28 at j=18..30 at
            # lag 2 (tile j-2 popped at j, so its exp-dependency never
            # blocks the PE queue), tiles 29+30 at j31 where nothing
            # queues behind them; tile 31 bypasses u (tail matmuls).
            pops = {3: 2, 4: 2, 5: 2, 31: 2}
            for j in list(range(6, 15)) + list(range(18, 31)):
                pops[j] = 1

            # ---- main loop ----
            dps0 = o0 = dps1 = o1 = a31 = None
            for j in range(2 * T):
                h, jl = j // T, j % T
                sc = psA.tile([P, HQ], F32, tag="sc", name=f"sc{j}")
                emit_scores(h, jl, sc)
                a_t = att.tile([P, HQ], BF16, tag="aT", name=f"a{j}")
                nc.scalar.activation(
                    a_t[:], sc[:], EXP,
                    bias=mask_bias[:, jl : jl + 1], scale=SCALE,
                )
                pop_u(pops.get(j, 0))
                if j == 1:
                    # qT2 half 1 on the psU chain, evacuated by DVE
                    pps1 = psU.tile([P, HQ], F32, tag="u", name="qp1")
                    for c in range(2):
                        nc.tensor.matmul(
                            pps1[:, c * 512 : (c + 1) * 512],
                            w2T_bf,
                            xq_cols(HQ + c * 512, HQ + (c + 1) * 512),
                            start=True,
                            stop=True,
                        )
                    nc.vector.tensor_copy(out=qT2[:, HQ:L], in_=pps1[:])
                elif j == 16:
                    # u0 evacuation (DVE; ACT is exp-bound), then vproj =
                    # V31 @ Wv on the psU chain so u1 frees right after.
                    # S15's add is deferred behind them (needed at j19).
                    nc.vector.tensor_copy(out=u_bf0[:], in_=u_ps[0][:])
                    vp = psU.tile([P, D], F32, tag="u", name="vp")
                    nc.tensor.matmul(
                        vp[:], vT31_sb, wv_bf, start=True, stop=True
                    )
                    nc.vector.tensor_copy(out=vproj[:], in_=vp[:])
                    emit_S(0, 15, a15)
                elif j == 19:
                    # h0 epilogue spread 4 matmuls/iter so no PE-queue
                    # burst delays the next scores tile
                    dps0 = psA.tile([P, HQ], F32, tag="sc", name="dps0")
                    den_mms(0, dps0, True, True, S_h[0], 4)
                elif j == 20:
                    for c in range(4, 8):
                        nc.tensor.matmul(
                            dps0[:, c : c + 1],
                            S_h[0][:, c * P : (c + 1) * P],
                            ones_col[:],
                            start=True,
                            stop=True,
                        )
                    den_recip(0, dps0)
                elif j == 21:
                    o0 = psA.tile([P, HQ], F32, tag="sc", name="o0")
                    o_mms(0, o0, 0, 4)
                elif j == 22:
                    o_mms(0, o0, 4, 8)
                    scale_out(0, o0, 0)
                elif j == 23:
                    scale_out(0, o0, 1)
                    out_dma(0, 0, nc.gpsimd)
                elif j == 24:
                    out_dma(0, 1, nc.gpsimd)
                elif j == 31:
                    # u1 closed at tile 30: both chunks evacuate on DVE
                    # during the last exp (chunk 0 first - it gates the
                    # first output pairs). Partial den over tiles 16..30
                    # (complete groups in dps1 bank A)
                    nc.vector.tensor_copy(
                        out=u1c[0][:], in_=u_ps[1][:, 0:512]
                    )
                    nc.vector.tensor_copy(
                        out=u1c[1][:], in_=u_ps[1][:, 512:HQ]
                    )
                    dps1 = psA.tile([P, HQ], F32, tag="sc", name="dps1")
                    den_mms(1, dps1, True, True, S_h[1], 8)
                    denTa = const.tile([P, 8], F32, tag="denTa")
                    nc.vector.tensor_copy(out=denTa[:], in_=dps1[:, 0:8])
                    o1a = psA.tile([P, HQ], F32, tag="sc", name="o1a")
                    o1b = psA.tile([P, HQ], F32, tag="sc", name="o1b")
                if j == 31:
                    a31 = a_t
                elif j == 15:
                    a15 = a_t  # S15-add deferred into the j16 branch
                    pend.append((h, jl, a_t))
                else:
                    emit_S(h, jl, a_t)
                    pend.append((h, jl, a_t))
                if j == 15:
                    pop_u(len(pend))  # close u(h0) before its epilogue
            assert not pend, f"unpopped u tiles: {len(pend)}"

            # ---- h1 tail: only tile-31 contributions remain ----
            # den finals on a31, complete groups in dps1 bank B
            for c in range(8):
                nc.tensor.matmul(
                    dps1[:, 640 + c : 641 + c],
                    a31[:, c * P : (c + 1) * P],
                    ones_col[:],
                    start=True,
                    stop=True,
                )
            denT[1] = const.tile([P, 8], F32, tag="denT1", name="denT1")
            nc.vector.tensor_tensor(
                denT[1][:], denTa[:], dps1[:, 640:648], ADD
            )
            rT[1] = const.tile([P, 8], F32, tag="rT1", name="rT1")
            nc.vector.reciprocal(rT[1][:], denT[1][:])
            # o1 per chunk: one complete group = a31 correction + u1 part;
            # chunks 0-3 in o1a, 4-7 in o1b (separate tiles so the g0
            # scale read does not block the 4-7 writes)
            for c in range(8):
                o_t = o1a if c < 4 else o1b
                cb = (c % 4) * P
                nc.tensor.matmul(
                    o_t[:, cb : cb + P],
                    a31[:, c * P : (c + 1) * P],
                    vproj[:],
                    start=True,
                    stop=False,
                )
                nc.tensor.matmul(
                    o_t[:, cb : cb + P],
                    u_bf_chunk(1, c),
                    wv_bf,
                    start=False,
                    stop=True,
                )
                if c == 3:
                    scale_out(1, o1a, 0, base=0)
                    out_dma(1, 0, nc.sync)
            scale_out(1, o1b, 1, base=0)
            # scalar queue (HWDGE, idle after the last exp): issues in
            # parallel with the g0 DMA on sync instead of queueing
            out_dma(1, 1, nc.scalar)

    nc.compile()
    return nc


_NC_CACHE = None


def _get_nc():
    global _NC_CACHE
    if _NC_CACHE is None:
        _NC_CACHE = build()
    return _NC_CACHE


def _prep_core_inputs(q_b, k_b, v_b, w2T, wv, mask_b):
    """Host-side layout prep for one core. q_b/k_b/v_b: [L, D] f32;
    w2T/wv: [D, D] bf16 (shared); mask_b: [L] int array."""
    mb = np.where(mask_b == 0, -30000.0, 0.0).astype(np.float32)
    mb_t = np.ascontiguousarray(mb.reshape(T, P).T)  # [P, 16] f32
    mb_bf = mb_t.view(BF16NP).reshape(P, 32)  # raw bytes as bf16 cols
    qT = q_b.T.astype(BF16NP)  # [128, 2048]
    kT = k_b.T.astype(BF16NP)
    vt = (
        v_b.reshape(T, P, D).transpose(1, 0, 2).reshape(P, L).astype(BF16NP)
    )  # [p, t*128+d] with k = t*128+p
    vT31 = np.ascontiguousarray(v_b[(T - 1) * P :, :].T.astype(BF16NP))
    return {
        "pack0": np.ascontiguousarray(
            np.concatenate([w2T, mb_bf, qT[:, 0:512]], axis=1)
        ),
        "pack1a": np.ascontiguousarray(
            np.concatenate(
                [qT[:, 512:HQ], kT[:, 0:512], vt[:, 0:512]], axis=1
            )
        ),
        "pack1b": np.ascontiguousarray(qT[:, HQ:L]),
        "pack2": np.ascontiguousarray(
            np.concatenate(
                [kT[:, 512:1536], vt[:, 512:1024], wv, vT31], axis=1
            )
        ),
        "pack3": np.ascontiguousarray(
            np.concatenate([kT[:, 1536:L], vt[:, 1024:L]], axis=1)
        ),
    }


def kernel(query, key, value, Wq, Wk, Wv, attention_mask):
    query = np.asarray(query, dtype=np.float32)
    key = np.asarray(key, dtype=np.float32)
    value = np.asarray(value, dtype=np.float32)
    Wq = np.asarray(Wq, dtype=np.float32)
    Wk = np.asarray(Wk, dtype=np.float32)
    Wv = np.asarray(Wv, dtype=np.float32)
    mask = np.asarray(attention_mask, dtype=np.int32).reshape(N_CORES, L)

    # fused scores weight: scores = (q Wq)(k Wk)^T = q (Wq Wk^T) k^T
    w2T = np.ascontiguousarray((Wq @ Wk.T).astype(BF16NP))
    wv = np.ascontiguousarray(Wv.astype(BF16NP))

    nc = _get_nc()
    in_maps = [
        _prep_core_inputs(query[b], key[b], value[b], w2T, wv, mask[b])
        for b in range(N_CORES)
    ]
    res = run_bass_kernel_spmd(nc, in_maps, core_ids=list(range(N_CORES)))
    out = np.stack(
        [
            np.asarray(res.results[b]["out"])
            .reshape(P, T, D)
            .transpose(1, 0, 2)
            .reshape(L, D)
            for b in range(N_CORES)
        ],
        axis=0,
    )
    return out.astype(np.float32)


if __name__ == "__main__":
    rng = np.random.default_rng(0)
    q = rng.standard_normal((N_CORES, L, D), dtype=np.float32)
    k = rng.standard_normal((N_CORES, L, D), dtype=np.float32)
    v = rng.standard_normal((N_CORES, L, D), dtype=np.float32)
    wq = rng.standard_normal((128, 128), dtype=np.float32) * 0.08
    wk = rng.standard_normal((128, 128), dtype=np.float32) * 0.08
    wv = rng.standard_normal((128, 128), dtype=np.float32) * 0.08
    m = np.ones((N_CORES, 1, L), dtype=np.int32)
    out = kernel(
        query=q, key=k, value=v, Wq=wq, Wk=wk, Wv=wv, attention_mask=m
    )
    print(out.shape, out.dtype)
